# revision 10
# baseline (speedup 1.0000x reference)
"""CQVAE loss kernel for Trainium2, data-parallel over batch on 8 NeuronCores.

loss = kld(qy) + mse(gather(rzs), zs[:, :Sg]) + bias(best, best_gt)
       + bias(gather(pts), gts)
where bias(p, g) = mse(p, g) + 10 * mse(p[..., MARK, :], g[..., MARK, :]).

Each core handles 16 of the 128 batches.  The mapping-gathers run as
multi-row indirect DMAs (hundreds of rows per op) so Q7 descriptor
emission never throttles the SWDGE queue.  pts/gts rows are zero-padded
to 256 floats on the host so gathered tiles line up on 1KB rows and pad
columns contribute nothing to the sums.  Each core ships a [128, 32]
per-partition stats tile; the host folds partitions and cores.
"""

import sys

import numpy as np

try:
    import concourse  # noqa: F401
except ImportError:  # pragma: no cover
    sys.path.insert(0, "/opt/trn_rl_repo")

import concourse.bass as bass  # noqa: F401
import concourse.mybir as mybir
import concourse.tile as tile
from concourse import bacc, library_config
from concourse.bass_utils import run_bass_kernel_spmd

F32 = mybir.dt.float32
I16 = mybir.dt.int16
AX = mybir.AxisListType
OP = mybir.AluOpType
ACTF = mybir.ActivationFunctionType

NCORES = 8
B, S, SG, D, P, V = 128, 256, 128, 1024, 118, 64
BL = B // NCORES  # batches per core
P2 = 2 * P  # 236 true floats per point-row
PC = 256  # padded point-row width
MARK = (0, 29, 88, 117)
EPS = 1e-20
ALPHA = 10.0

NSTAT = 32
# stats columns
C_BIAS0, C_BIAS1 = 0, 1  # bias sq totals per pts half
C_KLD = 2
C_BEST, C_BESTM = 3, 4
C_MARK0 = 5  # 4 cols: marks of half 0
C_MARK1 = 9  # 4 cols: marks of half 1
C_AE = 16  # 4 cols: ae pieces

KA = 4  # ae batches per piece (4 pieces)
KP = 8  # pts batches per half (2 halves)

_module = None
last_results = None  # BassKernelResults of the most recent run (for profiling)


def _build_module():
    nc = bacc.Bacc()

    zs = nc.dram_tensor("zs", [BL * SG, D], F32, kind="ExternalInput")
    rzs = nc.dram_tensor("rzs", [BL * S, D], F32, kind="ExternalInput")
    pts = nc.dram_tensor("pts", [BL * S, PC], F32, kind="ExternalInput")
    gts = nc.dram_tensor("gts", [BL * SG, PC], F32, kind="ExternalInput")
    qy = nc.dram_tensor("qy", [BL * S, V], F32, kind="ExternalInput")
    best = nc.dram_tensor("best", [BL, P2], F32, kind="ExternalInput")
    best_gt = nc.dram_tensor("best_gt", [BL, P2], F32, kind="ExternalInput")
    # dma_gather index lists, int16, wrapped [p, s] = lin[s*16 + p%16]:
    # cols 0..127   four rzs gathers (512 idxs each, 32 cols per op)
    # cols 128..255 two pts gathers (1024 idxs each, 64 cols per op)
    idx2 = nc.dram_tensor("idx2", [128, 256], I16, kind="ExternalInput")
    out = nc.dram_tensor("out", [128, NSTAT], F32, kind="ExternalOutput")

    QCOLS = BL * S * V // 128  # 2048
    QN = BL * S // 128  # 32 qy rows per partition

    with tile.TileContext(nc) as tc:
        with tc.tile_pool(name="cst", bufs=1) as cst:
            nc.gpsimd.load_library(library_config.mlp)
            idx_t = cst.tile([128, 256], I16)
            nc.sync.dma_start(idx_t[:], idx2[:])

            stats = cst.tile([128, NSTAT], F32)
            nc.vector.memset(stats[:], 0.0)

            # ---- direct loads --------------------------------------------
            # scalar HWDGE queue: qy, best, gts halves (3.2 MB)
            qy_t = cst.tile([128, QCOLS], F32)
            nc.scalar.dma_start(
                qy_t[:], qy[:].rearrange("(p n) v -> p (n v)", n=QN)
            )
            bt = cst.tile([BL, P2], F32)
            nc.scalar.dma_start(bt[:], best[:])
            bgt = cst.tile([BL, P2], F32)
            nc.scalar.dma_start(bgt[:], best_gt[:])
            gts_r = gts[:].rearrange("(p k) c -> p (k c)", k=BL)
            gt_h = []
            for h in range(2):
                g = cst.tile([128, KP * PC], F32, tag=f"gt{h}")
                nc.scalar.dma_start(g[:], gts_r[:, h * KP * PC : (h + 1) * KP * PC])
                gt_h.append(g)

            # sync HWDGE queue: zs pieces (8.4 MB)
            zs_r = zs[:].rearrange("(b s) d -> s b d", s=SG)
            zs_t = []
            for j in range(BL // KA):
                z = cst.tile([128, KA * D], F32, tag=f"zs{j}")
                nc.sync.dma_start(
                    z[:].rearrange("p (k d) -> p k d", d=D),
                    zs_r[:, j * KA : (j + 1) * KA, :],
                )
                zs_t.append(z)

            # ---- gathers (SWDGE queue, multi-row indirect) ----------------
            rg_t = []
            for j in range(4):
                rg = cst.tile([128, KA * D], F32, tag=f"rg{j}", name=f"rg{j}")
                rg_t.append(rg)
            pg_t = []
            for h in range(2):
                pg = cst.tile([128, KP * PC], F32, tag=f"pg{h}", name=f"pg{h}")
                pg_t.append(pg)

            def gather_rzs(j):
                nidx = KA * 128  # 512
                nc.gpsimd.dma_gather(
                    rg_t[j][:].rearrange("p (k d) -> p k d", d=D),
                    rzs[:],
                    idx_t[:, j * 32 : (j + 1) * 32],
                    nidx,
                    nidx,
                    D,
                )

            def gather_pts(h):
                nidx = KP * 128  # 1024
                nc.gpsimd.dma_gather(
                    pg_t[h][:].rearrange("p (k c) -> p k c", c=PC),
                    pts[:],
                    idx_t[:, 128 + h * 64 : 128 + (h + 1) * 64],
                    nidx,
                    nidx,
                    PC,
                )

            gather_rzs(0)
            gather_rzs(1)
            gather_pts(0)
            gather_rzs(2)
            gather_rzs(3)
            gather_pts(1)

            # ---- compute --------------------------------------------------
            # BEST (tiny, lands early on the scalar queue)
            nc.vector.tensor_sub(bt[:], bt[:], bgt[:])
            nc.vector.tensor_mul(bt[:], bt[:], bt[:])
            nc.vector.reduce_sum(out=stats[:BL, C_BEST : C_BEST + 1], in_=bt[:], axis=AX.X)
            bm4 = cst.tile([BL, 4], F32)
            for j, m in enumerate(MARK):
                nc.vector.reduce_sum(
                    out=bm4[:, j : j + 1], in_=bt[:, 2 * m : 2 * m + 2], axis=AX.X
                )
            nc.vector.reduce_sum(out=stats[:BL, C_BESTM : C_BESTM + 1], in_=bm4[:], axis=AX.X)

            # KLD: sum q * (log(q + eps) - log(1/V)) via log(V*q + V*eps)
            lg = cst.tile([128, QCOLS], F32)
            ebias = cst.tile([128, 1], F32)
            nc.vector.memset(ebias[:], float(V) * EPS)
            nc.scalar.activation(lg[:], qy_t[:], ACTF.Ln, bias=ebias[:], scale=float(V))
            nc.vector.scalar_tensor_tensor(
                out=lg[:],
                in0=lg[:],
                scalar=0.0,
                in1=qy_t[:],
                op0=OP.subtract,
                op1=OP.mult,
                accum_out=stats[:, C_KLD : C_KLD + 1],
            )

            def ae_piece(j):
                nc.vector.tensor_sub(rg_t[j][:], rg_t[j][:], zs_t[j][:])
                nc.scalar.activation(
                    rg_t[j][:], rg_t[j][:], ACTF.Square,
                    accum_out=stats[:, C_AE + j : C_AE + j + 1],
                )

            def bias_half(h, cbias, cmark):
                nc.vector.tensor_sub(pg_t[h][:], pg_t[h][:], gt_h[h][:])
                nc.scalar.activation(
                    pg_t[h][:], pg_t[h][:], ACTF.Square,
                    accum_out=stats[:, cbias : cbias + 1],
                )
                sq3 = pg_t[h][:].rearrange("p (k c) -> p k c", c=PC)
                for j, m in enumerate(MARK):
                    nc.vector.reduce_sum(
                        out=stats[:, cmark + j : cmark + j + 1],
                        in_=sq3[:, :, 2 * m : 2 * m + 2],
                        axis=AX.XY,
                    )

            ae_piece(0)
            ae_piece(1)
            bias_half(0, C_BIAS0, C_MARK0)
            ae_piece(2)
            ae_piece(3)
            bias_half(1, C_BIAS1, C_MARK1)

            nc.sync.dma_start(out[:], stats[:])

    nc.compile()
    return nc


def kernel(
    zs, rzs, pts, best, qy, gts, best_gt, mapping, vector_dims, **trace_kwargs
):
    global _module, last_results
    vd = int(np.asarray(vector_dims))
    assert vd == V, f"kernel compiled for vector_dims={V}, got {vd}"

    if _module is None:
        _module = _build_module()

    zs = np.asarray(zs, dtype=np.float32)
    rzs = np.asarray(rzs, dtype=np.float32)
    pts = np.asarray(pts, dtype=np.float32)
    gts = np.asarray(gts, dtype=np.float32)
    qy = np.asarray(qy, dtype=np.float32)
    mapping = np.asarray(mapping).astype(np.int32)
    best2 = np.ascontiguousarray(np.asarray(best, dtype=np.float32).reshape(B, P2))
    bgt2 = np.ascontiguousarray(np.asarray(best_gt, dtype=np.float32).reshape(B, P2))

    # zero-pad point rows to PC floats
    pts_p = np.zeros((B, S, PC), dtype=np.float32)
    pts_p[:, :, :P2] = pts.reshape(B, S, P2)
    gts_p = np.zeros((B, SG, PC), dtype=np.float32)
    gts_p[:, :, :P2] = gts.reshape(B, SG, P2)

    def wrap16(lin):
        # dma_gather index layout: idxs[p, s] = lin[s*16 + p%16]
        return np.tile(lin.reshape(-1, 16).T, (8, 1))

    kk512, pp512 = np.divmod(np.arange(4 * 128), 128)
    kk1024, pp1024 = np.divmod(np.arange(8 * 128), 128)
    in_maps = []
    for c in range(NCORES):
        sl = slice(c * BL, (c + 1) * BL)
        mp = mapping[sl]  # [BL, SG]
        blocks = []
        for j in range(4):  # rzs gathers: dst[p, k] = rzs[4j+k, mapping[4j+k, p]]
            b = 4 * j + kk512
            blocks.append(wrap16(b * S + mp[b, pp512]))
        for h in range(2):  # pts gathers: dst[p, k] matches gts rows 16p+8h+k
            b = pp1024 // 8
            pos = 16 * (pp1024 % 8) + 8 * h + kk1024
            blocks.append(wrap16(b * S + mp[b, pos]))
        idx2 = np.concatenate(blocks, axis=1).astype(np.int16)
        in_maps.append(
            {
                "zs": np.ascontiguousarray(
                    zs[sl, :SG].reshape(BL * SG, D)
                ),
                "rzs": rzs[sl].reshape(BL * S, D),
                "pts": pts_p[sl].reshape(BL * S, PC),
                "gts": gts_p[sl].reshape(BL * SG, PC),
                "qy": qy[sl].reshape(BL * S, V),
                "best": np.ascontiguousarray(best2[sl]),
                "best_gt": np.ascontiguousarray(bgt2[sl]),
                "idx2": np.ascontiguousarray(idx2),
            }
        )

    last_results = run_bass_kernel_spmd(
        _module, in_maps, list(range(NCORES)), **trace_kwargs
    )
    parts = np.stack(
        [
            np.asarray(r["out"], dtype=np.float64).reshape(128, NSTAT).sum(axis=0)
            for r in last_results.results
        ]
    )
    tot = parts.sum(axis=0)

    ae_loss = tot[C_AE : C_AE + 4].sum() / (B * SG * D)
    bias_sq = tot[C_BIAS0] + tot[C_BIAS1]
    mark_sq = tot[C_MARK0 : C_MARK0 + 4].sum() + tot[C_MARK1 : C_MARK1 + 4].sum()
    bias_loss = bias_sq / (B * SG * P2) + ALPHA * mark_sq / (B * SG * 2 * len(MARK))
    kld_loss = tot[C_KLD] / (B * S)
    best_mse = tot[C_BEST] / (B * P2) + ALPHA * tot[C_BESTM] / (B * 2 * len(MARK))

    return np.array(kld_loss + ae_loss + best_mse + bias_loss, dtype=np.float32)


# revision 11
# speedup vs baseline: 1.0260x; 1.0260x over previous
"""CQVAE loss kernel for Trainium2, data-parallel over batch on 8 NeuronCores.

loss = kld(qy) + mse(gather(rzs), zs[:, :Sg]) + bias(best, best_gt)
       + bias(gather(pts), gts)
where bias(p, g) = mse(p, g) + 10 * mse(p[..., MARK, :], g[..., MARK, :]).

Each core handles 16 of the 128 batches.  The mapping-gathers run as
dma_gather ops (hundreds of rows per op, ~9ns/row of Q7 emission)
interleaved so gather bytes, zs bytes and compute pipeline smoothly.
pts/gts rows are zero-padded to 256 floats on the host so gathered rows
are 1KB-aligned and pad columns contribute nothing to the sums.  zs/gts
are laid out so every partition reads one contiguous 64/16KB run.  Each
core ships a [128, 32] per-partition stats tile; the host folds
partitions and cores.
"""

import sys

import numpy as np

try:
    import concourse  # noqa: F401
except ImportError:  # pragma: no cover
    sys.path.insert(0, "/opt/trn_rl_repo")

import concourse.bass as bass  # noqa: F401
import concourse.mybir as mybir
import concourse.tile as tile
from concourse import bacc, library_config
from concourse.bass_utils import run_bass_kernel_spmd

F32 = mybir.dt.float32
I16 = mybir.dt.int16
AX = mybir.AxisListType
OP = mybir.AluOpType
ACTF = mybir.ActivationFunctionType

NCORES = 8
B, S, SG, D, P, V = 128, 256, 128, 1024, 118, 64
BL = B // NCORES  # batches per core
P2 = 2 * P  # 236 true floats per point-row
PC = 256  # padded point-row width
MARK = (0, 29, 88, 117)
EPS = 1e-20
ALPHA = 10.0

NSTAT = 32
# stats columns
C_KLD = 28
C_BEST, C_BESTM = 29, 30
C_AE = 0  # 8 cols: ae pieces
C_BIAS = 8  # 4 cols: bias sq totals per pts quarter
C_MARK = 12  # 16 cols: 4 marks x 4 quarters

NAE = 8  # rzs gather ops / zs pieces (2 batches each)
NPT = 4  # pts gather ops / gts quarters (4 batches each)
KA = BL // NAE  # 2 batch-slots per ae piece
KP = BL // NPT  # 4 batch-slots per pts quarter

_module = None
last_results = None  # BassKernelResults of the most recent run (for profiling)


def _build_module():
    nc = bacc.Bacc()

    zs = nc.dram_tensor("zs", [BL * SG, D], F32, kind="ExternalInput")
    rzs = nc.dram_tensor("rzs", [BL * S, D], F32, kind="ExternalInput")
    pts = nc.dram_tensor("pts", [BL * S, PC], F32, kind="ExternalInput")
    gts = nc.dram_tensor("gts", [BL * SG, PC], F32, kind="ExternalInput")
    qy = nc.dram_tensor("qy", [BL * S, V], F32, kind="ExternalInput")
    best = nc.dram_tensor("best", [BL, P2], F32, kind="ExternalInput")
    best_gt = nc.dram_tensor("best_gt", [BL, P2], F32, kind="ExternalInput")
    # dma_gather index lists, int16, wrapped [p, s] = lin[s*16 + p%16]:
    # cols 0..127   eight rzs gathers (256 idxs each, 16 cols per op)
    # cols 128..255 four pts gathers (512 idxs each, 32 cols per op)
    idx2 = nc.dram_tensor("idx2", [128, 256], I16, kind="ExternalInput")
    out = nc.dram_tensor("out", [128, NSTAT], F32, kind="ExternalOutput")

    QCOLS = BL * S * V // 128  # 2048
    QN = BL * S // 128  # 32 qy rows per partition

    with tile.TileContext(nc) as tc:
        with tc.tile_pool(name="cst", bufs=1) as cst:
            nc.gpsimd.load_library(library_config.mlp)
            idx_t = cst.tile([128, 256], I16)
            nc.sync.dma_start(idx_t[:], idx2[:])

            stats = cst.tile([128, NSTAT], F32)
            nc.vector.memset(stats[:], 0.0)

            # ---- direct loads --------------------------------------------
            # scalar HWDGE queue: qy, best, gts quarters (3.2 MB)
            qy_t = cst.tile([128, QCOLS], F32)
            nc.scalar.dma_start(
                qy_t[:], qy[:].rearrange("(p n) v -> p (n v)", n=QN)
            )
            bt = cst.tile([BL, P2], F32)
            nc.scalar.dma_start(bt[:], best[:])
            bgt = cst.tile([BL, P2], F32)
            nc.scalar.dma_start(bgt[:], best_gt[:])
            # partition p holds gts rows 16p..16p+15 (contiguous 16KB)
            gts_r = gts[:].rearrange("(p k) c -> p (k c)", k=BL)
            gt_h = []
            for h in range(NPT):
                g = cst.tile([128, KP * PC], F32, tag=f"gt{h}", name=f"gt{h}")
                nc.scalar.dma_start(g[:], gts_r[:, h * KP * PC : (h + 1) * KP * PC])
                gt_h.append(g)

            # sync HWDGE queue: zs pieces (8.4 MB)
            # partition p holds zs rows 16p..16p+15 (contiguous 64KB)
            zs_r = zs[:].rearrange("(p k) d -> p (k d)", k=BL)
            zs_t = []
            for j in range(NAE):
                z = cst.tile([128, KA * D], F32, tag=f"zs{j}", name=f"zs{j}")
                nc.sync.dma_start(z[:], zs_r[:, j * KA * D : (j + 1) * KA * D])
                zs_t.append(z)

            # ---- gathers (SWDGE queue 0, dma_gather) ----------------------
            rg_t = []
            for j in range(NAE):
                rg = cst.tile([128, KA * D], F32, tag=f"rg{j}", name=f"rg{j}")
                rg_t.append(rg)
            pg_t = []
            for h in range(NPT):
                pg = cst.tile([128, KP * PC], F32, tag=f"pg{h}", name=f"pg{h}")
                pg_t.append(pg)

            def gather_rzs(j):
                nidx = KA * 128  # 256
                nc.gpsimd.dma_gather(
                    rg_t[j][:].rearrange("p (k d) -> p k d", d=D),
                    rzs[:],
                    idx_t[:, j * 16 : (j + 1) * 16],
                    nidx,
                    nidx,
                    D,
                )

            def gather_pts(h):
                nidx = KP * 128  # 512
                nc.gpsimd.dma_gather(
                    pg_t[h][:].rearrange("p (k c) -> p k c", c=PC),
                    pts[:],
                    idx_t[:, 128 + h * 32 : 128 + (h + 1) * 32],
                    nidx,
                    nidx,
                    PC,
                )

            for h in range(NPT):
                gather_rzs(2 * h)
                gather_rzs(2 * h + 1)
                gather_pts(h)

            # ---- compute --------------------------------------------------
            # BEST (tiny, lands early on the scalar queue)
            nc.vector.tensor_sub(bt[:], bt[:], bgt[:])
            nc.vector.tensor_mul(bt[:], bt[:], bt[:])
            nc.vector.reduce_sum(out=stats[:BL, C_BEST : C_BEST + 1], in_=bt[:], axis=AX.X)
            bm4 = cst.tile([BL, 4], F32)
            for j, m in enumerate(MARK):
                nc.vector.reduce_sum(
                    out=bm4[:, j : j + 1], in_=bt[:, 2 * m : 2 * m + 2], axis=AX.X
                )
            nc.vector.reduce_sum(out=stats[:BL, C_BESTM : C_BESTM + 1], in_=bm4[:], axis=AX.X)

            # KLD: sum q * (log(q + eps) - log(1/V)) via log(V*q + V*eps)
            lg = cst.tile([128, QCOLS], F32)
            ebias = cst.tile([128, 1], F32)
            nc.vector.memset(ebias[:], float(V) * EPS)
            nc.scalar.activation(lg[:], qy_t[:], ACTF.Ln, bias=ebias[:], scale=float(V))
            nc.vector.scalar_tensor_tensor(
                out=lg[:],
                in0=lg[:],
                scalar=0.0,
                in1=qy_t[:],
                op0=OP.subtract,
                op1=OP.mult,
                accum_out=stats[:, C_KLD : C_KLD + 1],
            )

            def ae_piece(j):
                nc.vector.tensor_sub(rg_t[j][:], rg_t[j][:], zs_t[j][:])
                nc.scalar.activation(
                    rg_t[j][:], rg_t[j][:], ACTF.Square,
                    accum_out=stats[:, C_AE + j : C_AE + j + 1],
                )

            def bias_quarter(h):
                nc.vector.tensor_sub(pg_t[h][:], pg_t[h][:], gt_h[h][:])
                nc.scalar.activation(
                    pg_t[h][:], pg_t[h][:], ACTF.Square,
                    accum_out=stats[:, C_BIAS + h : C_BIAS + h + 1],
                )
                sq3 = pg_t[h][:].rearrange("p (k c) -> p k c", c=PC)
                cm = C_MARK + 4 * h
                for j, m in enumerate(MARK):
                    nc.vector.reduce_sum(
                        out=stats[:, cm + j : cm + j + 1],
                        in_=sq3[:, :, 2 * m : 2 * m + 2],
                        axis=AX.XY,
                    )

            for h in range(NPT):
                ae_piece(2 * h)
                ae_piece(2 * h + 1)
                bias_quarter(h)

            nc.sync.dma_start(out[:], stats[:])

    nc.compile()
    return nc


def kernel(
    zs, rzs, pts, best, qy, gts, best_gt, mapping, vector_dims, **trace_kwargs
):
    global _module, last_results
    vd = int(np.asarray(vector_dims))
    assert vd == V, f"kernel compiled for vector_dims={V}, got {vd}"

    if _module is None:
        _module = _build_module()

    zs = np.asarray(zs, dtype=np.float32)
    rzs = np.asarray(rzs, dtype=np.float32)
    pts = np.asarray(pts, dtype=np.float32)
    gts = np.asarray(gts, dtype=np.float32)
    qy = np.asarray(qy, dtype=np.float32)
    mapping = np.asarray(mapping).astype(np.int32)
    best2 = np.ascontiguousarray(np.asarray(best, dtype=np.float32).reshape(B, P2))
    bgt2 = np.ascontiguousarray(np.asarray(best_gt, dtype=np.float32).reshape(B, P2))

    # zero-pad point rows to PC floats
    pts_p = np.zeros((B, S, PC), dtype=np.float32)
    pts_p[:, :, :P2] = pts.reshape(B, S, P2)
    gts_p = np.zeros((B, SG, PC), dtype=np.float32)
    gts_p[:, :, :P2] = gts.reshape(B, SG, P2)

    def wrap16(lin):
        # dma_gather index layout: idxs[p, s] = lin[s*16 + p%16]
        return np.tile(lin.reshape(-1, 16).T, (8, 1))

    # partition p <-> (b = p//8, q = p%8); slot k within a piece
    kk_a, pp_a = np.divmod(np.arange(KA * 128), 128)  # rzs ops
    kk_p, pp_p = np.divmod(np.arange(KP * 128), 128)  # pts ops
    in_maps = []
    for c in range(NCORES):
        sl = slice(c * BL, (c + 1) * BL)
        mp = mapping[sl]  # [BL, SG]
        blocks = []
        for j in range(NAE):  # dst[p, k] = rzs[b, mapping[b, 16q + KA*j + k]]
            b = pp_a // 8
            pos = 16 * (pp_a % 8) + KA * j + kk_a
            blocks.append(wrap16(b * S + mp[b, pos]))
        for h in range(NPT):  # dst[p, k] matches gts rows 16p + KP*h + k
            b = pp_p // 8
            pos = 16 * (pp_p % 8) + KP * h + kk_p
            blocks.append(wrap16(b * S + mp[b, pos]))
        idx2 = np.concatenate(blocks, axis=1).astype(np.int16)
        # zs rows reordered so partition p holds rows 16p..16p+15:
        # row 16p+k = zs[b, 16q+k] -> natural order already (b-major, i-minor)
        in_maps.append(
            {
                "zs": np.ascontiguousarray(zs[sl, :SG].reshape(BL * SG, D)),
                "rzs": rzs[sl].reshape(BL * S, D),
                "pts": pts_p[sl].reshape(BL * S, PC),
                "gts": gts_p[sl].reshape(BL * SG, PC),
                "qy": qy[sl].reshape(BL * S, V),
                "best": np.ascontiguousarray(best2[sl]),
                "best_gt": np.ascontiguousarray(bgt2[sl]),
                "idx2": np.ascontiguousarray(idx2),
            }
        )

    last_results = run_bass_kernel_spmd(
        _module, in_maps, list(range(NCORES)), **trace_kwargs
    )
    parts = np.stack(
        [
            np.asarray(r["out"], dtype=np.float64).reshape(128, NSTAT).sum(axis=0)
            for r in last_results.results
        ]
    )
    tot = parts.sum(axis=0)

    ae_loss = tot[C_AE : C_AE + NAE].sum() / (B * SG * D)
    bias_sq = tot[C_BIAS : C_BIAS + NPT].sum()
    mark_sq = tot[C_MARK : C_MARK + 4 * NPT].sum()
    bias_loss = bias_sq / (B * SG * P2) + ALPHA * mark_sq / (B * SG * 2 * len(MARK))
    kld_loss = tot[C_KLD] / (B * S)
    best_mse = tot[C_BEST] / (B * P2) + ALPHA * tot[C_BESTM] / (B * 2 * len(MARK))

    return np.array(kld_loss + ae_loss + best_mse + bias_loss, dtype=np.float32)
